# revision 1
# baseline (speedup 1.0000x reference)
"""BoxPairMultiScaleRoIAlign Trainium2 kernel (8 NeuronCores, SPMD).

Approach
--------
The reference computes, per box-pair k: a masked RoIAlign of the union box
over one FPN level (chosen by the torchvision LevelMapper), where the pair
mask is max(union_mask, object_mask).  Because the object box is contained
in the union box, per-axis pixel coverage is pointwise monotone in interval
inclusion, so max(mask_u, mask_o) == mask_u — the whole weighting is a
separable product of a per-row factor and a per-column factor.  Each box's
[256, 7, 7] output is therefore an exact linear map of a small pixel grid
Y x X (|Y|,|X| <= 28) of its level's feature map:

    out[c, (py,px)] = sum_{yi,xi} Ay[yi,py] * Ax[xi,px] * feat[c, Y[yi], X[xi]]

The host (numpy) computes levels, pixel grids and the weight matrices
W = Ay (x) Ax exactly in float32; the device does the memory-heavy part:
indirect-DMA gathers of the pixel rows from a channels-last feature table
(bf16), and PE matmuls  psum[128c, 49] += G[128pix, 128c].T @ W[128pix, 49]
accumulated per box, then batched stores.

Sharding: data-parallel over K.  Boxes are snake-dealt by descending tile
count to the 8 cores (64 slots/core), so every core runs an identical
static schedule (slot-wise tile counts padded to the max across cores).
"""

import os
import numpy as np
import ml_dtypes

import concourse.bass as bass
import concourse.bacc as bacc
import concourse.mybir as mybir
from concourse.tile import TileContext
from concourse.bass_utils import run_bass_kernel_spmd

# ----------------------------------------------------------------------------
# problem constants (hardcoded; kernel.py must be self-contained)
P, SR = 7, 2
NBINS = P * P
SCALES = (0.25, 0.125, 0.0625, 0.03125)
LVL_HW = ((128, 128), (64, 64), (32, 32), (16, 16))
K_MIN, K_MAX = 2.0, 5.0
LVL0, S0, EPS = 4.0, 224.0, 1e-6
C = 256
N_IMG = 2
NCORES = 8
KSLOTS = 64          # boxes per core (512 / 8)

RUN = int(os.environ.get("ROI_RUN", "8"))   # consecutive x-pixels per gather index
CW = int(os.environ.get("ROI_CW", str(max(2, 128 // RUN))))
GBUFS = int(os.environ.get("ROI_GBUFS", "8"))
WBUFS = int(os.environ.get("ROI_WBUFS", "4"))
PBUFS = int(os.environ.get("ROI_PBUFS", "6"))
WENG = os.environ.get("ROI_WENG", "sync")   # engine ring for W-chunk loads
GS = 16              # slots per output store group
TABLE_ROWS = sum(N_IMG * h * w for h, w in LVL_HW)  # 43520
TABLE_PAD = TABLE_ROWS + 8   # runs may read past the last row; pad with zeros

f32 = np.float32
BF16 = ml_dtypes.bfloat16

# ----------------------------------------------------------------------------
# host-side planning (exact float32 reference math)
def _coverage_vals(x1, x2, idx):
    """Coverage of clipped interval [x1,x2] at integer pixel idx (f32)."""
    x1f = np.floor(x1)
    x2c = np.ceil(x2)
    inside = ((idx >= x1f) & (idx < x2c)).astype(f32)
    wl = np.where(idx == x1f, f32(1.0) + x1f - x1, f32(1.0)).astype(f32)
    wr = np.where(idx == x2c - f32(1.0), f32(1.0) + x2 - x2c, f32(1.0)).astype(f32)
    return inside * wl * wr


def _axis_interp(coord, size):
    valid = ((coord >= f32(-1.0)) & (coord <= f32(size))).astype(f32)
    c = np.maximum(coord, f32(0.0))
    lo0 = np.floor(c)
    top = lo0 >= f32(size - 1)
    lo = np.where(top, f32(size - 1), lo0).astype(f32)
    hi = np.where(top, f32(size - 1), lo0 + f32(1.0)).astype(f32)
    c = np.where(top, f32(size - 1), c).astype(f32)
    frac = (c - lo).astype(f32)
    wl = ((f32(1.0) - frac) * valid).astype(f32)
    wh = (frac * valid).astype(f32)
    return lo.astype(np.int32), hi.astype(np.int32), wl, wh


def _plan(boxes_h, boxes_o):
    """Per box: (level, image, Y, X, Ay [|Y|,7], Ax [|X|,7])."""
    K = boxes_h.shape[0]
    bidx = boxes_h[:, 0].astype(np.int32)
    bh = boxes_h[:, 1:].astype(f32)
    bo = boxes_o[:, 1:].astype(f32)
    ub = np.stack(
        [np.minimum(bh[:, 0], bo[:, 0]), np.minimum(bh[:, 1], bo[:, 1]),
         np.maximum(bh[:, 2], bo[:, 2]), np.maximum(bh[:, 3], bo[:, 3])],
        axis=1).astype(f32)
    su = np.sqrt(((ub[:, 2] - ub[:, 0]) * (ub[:, 3] - ub[:, 1])).astype(f32)).astype(f32)
    so = np.sqrt(((bo[:, 2] - bo[:, 0]) * (bo[:, 3] - bo[:, 1])).astype(f32)).astype(f32)
    s = np.minimum(su, so)
    lv = (np.clip(np.floor(LVL0 + np.log2(s / f32(S0) + f32(EPS))), K_MIN, K_MAX)
          ).astype(np.int32) - int(K_MIN)

    i = np.arange(P * SR)
    t = ((i // SR).astype(f32) + ((i % SR).astype(f32) + f32(0.5)) / f32(SR)).astype(f32)

    plans = []
    for k in range(K):
        l = int(lv[k]); H, W = LVL_HW[l]; sc = f32(SCALES[l])
        ubs = (ub[k] * sc).astype(f32)
        x1, y1, x2, y2 = ubs
        ux1 = np.clip(x1, 0.0, W).astype(f32); uy1 = np.clip(y1, 0.0, H).astype(f32)
        ux2 = np.clip(x2, 0.0, W).astype(f32); uy2 = np.clip(y2, 0.0, H).astype(f32)
        roi_w = np.maximum(x2 - x1, f32(1.0)).astype(f32)
        roi_h = np.maximum(y2 - y1, f32(1.0)).astype(f32)
        px = (x1 + t * (roi_w / f32(P))).astype(f32)
        py = (y1 + t * (roi_h / f32(P))).astype(f32)
        xlo, xhi, wxl, wxh = _axis_interp(px, W)
        ylo, yhi, wyl, wyh = _axis_interp(py, H)
        vx = np.stack([wxl * _coverage_vals(ux1, ux2, xlo.astype(f32)),
                       wxh * _coverage_vals(ux1, ux2, xhi.astype(f32))], axis=1).astype(f32)
        vy = np.stack([wyl * _coverage_vals(uy1, uy2, ylo.astype(f32)),
                       wyh * _coverage_vals(uy1, uy2, yhi.astype(f32))], axis=1).astype(f32)
        xs = np.stack([xlo, xhi], axis=1)
        ys = np.stack([ylo, yhi], axis=1)
        X = np.unique(xs); Y = np.unique(ys)
        Ax = np.zeros((len(X), P), dtype=f32)
        Ay = np.zeros((len(Y), P), dtype=f32)
        xpos = np.searchsorted(X, xs)
        ypos = np.searchsorted(Y, ys)
        half = f32(0.5)
        for smp in range(P * SR):
            b = smp // SR
            for a in range(2):
                Ax[xpos[smp, a], b] += vx[smp, a] * half
                Ay[ypos[smp, a], b] += vy[smp, a] * half
        plans.append((l, int(bidx[k]), Y, X, Ay, Ax))
    return plans


def _box_runs(plan):
    """Run-cover of a box's pixel grid: gather indices (run starts, y-major)
    and per-run weights [nruns, RUN, 49]."""
    l, n, Y, X, Ay, Ax = plan
    H, W = LVL_HW[l]
    bases = _LEVEL_BASES
    Xs = np.sort(X)
    starts = []
    i = 0
    while i < len(Xs):
        s = int(Xs[i])
        starts.append(s)
        while i < len(Xs) and Xs[i] < s + RUN:
            i += 1
    S = np.array(starts, dtype=np.int64)
    NS = len(S)
    # per (run, r) x-weight rows: Ax row of x = S+r when x in X else 0
    Axr = np.zeros((NS, RUN, P), dtype=f32)
    xpos = {int(x): i for i, x in enumerate(X)}
    for si in range(NS):
        for r in range(RUN):
            xi = xpos.get(int(S[si]) + r)
            if xi is not None:
                Axr[si, r] = Ax[xi]
    idx = (bases[l] + (n * H + Y[:, None]) * W + S[None, :]).ravel()  # [|Y|*NS]
    Wr = (Ay[:, None, None, :, None] * Axr[None, :, :, None, :]
          ).reshape(len(Y) * NS, RUN, NBINS).astype(f32)
    return idx.astype(np.int32), Wr


_LEVEL_BASES = np.cumsum([0] + [N_IMG * h * w for h, w in LVL_HW])[:4]


def _build_schedule(plans):
    """Snake-deal boxes to cores; build identical per-core job structure.

    Returns (cnt, slot_map, per_core_idx, per_core_w):
      cnt[s]           jobs (128-run gathers) for slot s, same on every core
      slot_map[c][s]   original box index handled by core c slot s
      per_core_idx[c]  int32 [128, T] gather indices (run starts)
      per_core_w[c]    bf16  [128, T*RUN*49] weights
    """
    runs_w = [_box_runs(p) for p in plans]
    ntiles = np.array([(len(rw[0]) + 127) // 128 for rw in runs_w])
    order = np.argsort(-ntiles, kind="stable")
    slot_map = np.zeros((NCORES, KSLOTS), dtype=np.int64)
    for rank, k in enumerate(order):
        row, pos = divmod(rank, NCORES)
        c = pos if row % 2 == 0 else NCORES - 1 - pos
        slot_map[c, row] = k
    cnt = [int(max(ntiles[slot_map[c, s]] for c in range(NCORES)))
           for s in range(KSLOTS)]
    T = sum(cnt)
    pad = (-T) % CW
    cnt[-1] += pad
    T += pad

    per_core_idx, per_core_w = [], []
    for c in range(NCORES):
        gidx = np.zeros((T, 128), dtype=np.int32)
        wmat = np.zeros((T, 128, RUN * NBINS), dtype=f32)
        jj = 0
        for s in range(KSLOTS):
            idx, Wr = runs_w[slot_map[c, s]]
            nruns = idx.shape[0]
            gidx[jj:jj + (nruns + 127) // 128].reshape(-1)[:nruns] = idx
            wmat[jj:jj + (nruns + 127) // 128].reshape(-1, RUN * NBINS)[:nruns] = (
                Wr.reshape(nruns, RUN * NBINS))
            jj += cnt[s]
        per_core_idx.append(np.ascontiguousarray(gidx.T))
        per_core_w.append(np.ascontiguousarray(
            wmat.transpose(1, 0, 2).reshape(128, T * RUN * NBINS)).astype(BF16))
    return cnt, slot_map, per_core_idx, per_core_w


# ----------------------------------------------------------------------------
# device program
def _build_nc(cnt, repeat=1):
    """Build the Bass program for one core given per-slot tile counts.

    repeat > 1 statically repeats the whole body (same inputs/outputs) —
    used only for slope-based wall-clock timing of the NEFF body.
    """
    T = sum(cnt)
    WJ = RUN * NBINS                       # weight columns per job
    nc = bacc.Bacc(None, target_bir_lowering=False)
    table = nc.declare_dram_parameter("table", [TABLE_PAD, C], mybir.dt.bfloat16,
                                      isOutput=False)
    gidx = nc.declare_dram_parameter("gidx", [128, T], mybir.dt.int32,
                                     isOutput=False)
    wmat = nc.declare_dram_parameter("wmat", [128, T * WJ], mybir.dt.bfloat16,
                                     isOutput=False)
    out = nc.declare_dram_parameter("out", [2, 128, KSLOTS * NBINS],
                                    mybir.dt.float32, isOutput=True)

    # slot -> (first job, #jobs)
    starts = np.concatenate([[0], np.cumsum(cnt)]).astype(int)

    with TileContext(nc) as tc:
        with (
            tc.tile_pool(name="const", bufs=1) as cpool,
            tc.tile_pool(name="gather", bufs=GBUFS) as gpool,
            tc.tile_pool(name="wts", bufs=WBUFS) as wpool,
            tc.tile_pool(name="obuf", bufs=2) as opool,
            tc.tile_pool(name="psum", bufs=PBUFS, space="PSUM") as ppool,
        ):
            gidx_t = cpool.tile([128, T], mybir.dt.int32)
            nc.sync.dma_start(out=gidx_t[:], in_=gidx[:])

            for rep in range(repeat):
              wtiles = {}
              obuf = None
              for s in range(KSLOTS):
                j0, njobs = starts[s], cnt[s]
                if s % GS == 0:
                    obuf = opool.tile([128, 2 * GS * NBINS], mybir.dt.float32,
                                      tag="obuf", name=f"ob{rep}_{s}")
                pb = [ppool.tile([128, NBINS], mybir.dt.float32, tag="pb",
                                 name=f"pb{rep}_{s}_{h}")
                      for h in range(2)]
                for t in range(njobs):
                    jj = j0 + t
                    gt = gpool.tile([128, RUN * C], mybir.dt.bfloat16,
                                    tag="g", name=f"g{rep}_{jj}")
                    nc.gpsimd.indirect_dma_start(
                        out=gt[:],
                        out_offset=None,
                        in_=table[:],
                        in_offset=bass.IndirectOffsetOnAxis(
                            ap=gidx_t[:, jj:jj + 1], axis=0),
                    )
                    w = jj // CW
                    if w not in wtiles:
                        wt = wpool.tile([128, CW * WJ], mybir.dt.bfloat16,
                                        tag="w", name=f"w{rep}_{w}")
                        getattr(nc, WENG).dma_start(
                            out=wt[:],
                            in_=wmat[:, w * CW * WJ:(w + 1) * CW * WJ])
                        wtiles[w] = wt
                    wt = wtiles[w]
                    for r in range(RUN):
                        wslice = wt[:, (jj - w * CW) * WJ + r * NBINS:
                                    (jj - w * CW) * WJ + (r + 1) * NBINS]
                        for h in range(2):
                            nc.tensor.matmul(
                                out=pb[h][:],
                                lhsT=gt[:, r * C + h * 128: r * C + h * 128 + 128],
                                rhs=wslice,
                                start=(t == 0 and r == 0),
                                stop=(t == njobs - 1 and r == RUN - 1),
                            )
                for h in range(2):
                    nc.vector.tensor_copy(
                        out=obuf[:, (h * GS + (s % GS)) * NBINS:
                                 (h * GS + (s % GS) + 1) * NBINS],
                        in_=pb[h][:])
                if s % GS == GS - 1:
                    g0 = s - (GS - 1)
                    for h in range(2):
                        nc.sync.dma_start(
                            out=out[h, :, g0 * NBINS:(g0 + GS) * NBINS],
                            in_=obuf[:, h * GS * NBINS:(h + 1) * GS * NBINS])
    nc.compile()
    return nc


_CACHE = {}


def _get_nc(cnt):
    key = tuple(cnt)
    if key not in _CACHE:
        _CACHE[key] = _build_nc(cnt)
    return _CACHE[key]


# results of the last device run, for test harnesses to inspect
LAST_RESULTS = None


def kernel(feat0, feat1, feat2, feat3, boxes_h, boxes_o):
    global LAST_RESULTS
    feats = [np.asarray(f, dtype=f32) for f in (feat0, feat1, feat2, feat3)]
    boxes_h = np.asarray(boxes_h, dtype=f32)
    boxes_o = np.asarray(boxes_o, dtype=f32)
    K = boxes_h.shape[0]

    # channels-last flat bf16 feature table (zero-padded tail), replicated
    table = np.zeros((TABLE_PAD, C), dtype=BF16)
    table[:TABLE_ROWS] = np.concatenate(
        [f.transpose(0, 2, 3, 1).reshape(-1, C) for f in feats],
        axis=0).astype(BF16)

    plans = _plan(boxes_h, boxes_o)
    cnt, slot_map, per_core_idx, per_core_w = _build_schedule(plans)
    nc = _get_nc(cnt)

    in_maps = [
        {"table": table, "gidx": per_core_idx[c], "wmat": per_core_w[c]}
        for c in range(NCORES)
    ]
    res = run_bass_kernel_spmd(nc, in_maps, core_ids=list(range(NCORES)))
    LAST_RESULTS = res

    out = np.empty((K, C, P, P), dtype=f32)
    for c in range(NCORES):
        oc = res.results[c]["out"]          # [2, 128, 64*49]
        oc = oc.reshape(2, 128, KSLOTS, NBINS)
        for s in range(KSLOTS):
            out[slot_map[c, s]] = oc[:, :, s, :].reshape(C, P, P)
    return out

